# revision 1
# baseline (speedup 1.0000x reference)
"""Distributed Bass kernel for a 4-layer GAT autoencoder on 8 TRN2 NeuronCores.

Strategy (per sharding hint): nodes sharded across 8 cores (2500/core);
edges co-located with their destination node's core, sorted by destination;
params replicated. Node-level compute (x@W, attention score projections) is
replicated on every core (cheap); the edge phase (gather + segment softmax +
weighted scatter) is sharded by destination. Layer outputs are AllGathered
in-kernel so the next layer's node phase sees the full node set.

Edge phase per 128-edge chunk:
  - dma_gather source-node rows [h | s_src] from a DRAM table (by src id)
  - dma_gather destination rows [s_dst] (by dst id)
  - e = LeakyReLU(s_src + s_dst); ex = exp(e)  (softmax max-shift dropped:
    exp ratios are shift-invariant and |e| is small for this data)
  - one-hot matrix O[e,d] = (dstloc[e]==d) built via is_equal vs an iota tile
  - PSUM += O.T @ [h*ex | ex]  accumulates both numerator and denominator
  - after all chunks: out = num/den (+1e-16), bias, head-concat or head-mean
"""

import sys

sys.path.insert(0, "/opt/trn_rl_repo")

import numpy as np

P = 128
M = 8
N = 20000
NPC = N // M  # 2500 nodes per core
NT = (NPC + P - 1) // P  # 20 dst tiles per core
HEADS = 8
NEG = 0.2
BN_EPS = 1e-5
DUMMY = N  # dummy table row for pad edges
NEGBIG = -1.0e30

# layer configs: Fin, C (per-head out), concat?, bn on input?, table row width R
LAYERS = [
    dict(Fin=64, C=16, concat=True, bn=False, R=192),
    dict(Fin=128, C=32, concat=False, bn=True, R=320),
    dict(Fin=32, C=16, concat=True, bn=False, R=192),
    dict(Fin=128, C=64, concat=False, bn=True, R=576),
]
NODE_TILES = (N + P - 1) // P  # 157 (last has 32 rows)


def _wrap16(idx):
    """Host int array -> dma_gather index layout [16, n/16] (idx[s*16+p] at [p,s])."""
    n = idx.shape[0]
    assert n % 16 == 0
    w = np.ascontiguousarray(idx.reshape(n // 16, 16).T).astype(np.int16)
    return np.ascontiguousarray(np.tile(w, (8, 1)))  # replicated for the 8 Q7 cores


def _preprocess(edge_index):
    """Partition + sort edges; build per-core gather/index arrays."""
    src = np.concatenate([np.asarray(edge_index[0]), np.arange(N)]).astype(np.int64)
    dst = np.concatenate([np.asarray(edge_index[1]), np.arange(N)]).astype(np.int64)

    per_core = []
    max_cnt = 0
    for m in range(M):
        sel = (dst // NPC) == m
        s, d = src[sel], dst[sel]
        dloc = d - NPC * m
        order = np.argsort(dloc, kind="stable")
        s, dloc = s[order], dloc[order]
        tiles = []
        for t in range(NT):
            tsel = (dloc // P) == t
            st, dt_ = s[tsel], dloc[tsel] - t * P
            tiles.append((st, dt_))
            max_cnt = max(max_cnt, st.shape[0])
        per_core.append(tiles)

    NCH = (max_cnt + P - 1) // P  # chunks per dst tile (uniform across cores)
    NCH = ((NCH + 3) // 4) * 4  # idx segments 64B-aligned for dma_gather ucode
    EPT = NCH * P  # padded edges per dst tile

    data = []
    for m in range(M):
        isrc = np.full((NT, EPT), DUMMY, dtype=np.int64)
        idst = np.full((NT, EPT), DUMMY, dtype=np.int64)
        dloc = np.full((NT, EPT), P - 1, dtype=np.int64)
        for t, (st, dt_) in enumerate(per_core[m]):
            c = st.shape[0]
            isrc[t, :c] = st
            idst[t, :c] = NPC * m + t * P + dt_
            dloc[t, :c] = dt_
        # dstloc layout [128, NT*NCH]: column t*NCH+j, row p = edge (t, j*128+p)
        dstloc = (
            dloc.reshape(NT, NCH, P).transpose(2, 0, 1).reshape(P, NT * NCH)
        ).astype(np.float32)
        data.append(
            dict(
                idx_src=_wrap16(isrc.reshape(-1)),
                idx_dst=_wrap16(idst.reshape(-1)),
                dstloc=np.ascontiguousarray(dstloc),
            )
        )
    return NCH, data


def _host_consts(inputs):
    """Fused weights + broadcast biases + misc consts (all replicated)."""
    f32 = np.float32
    c = {}
    c["iotaf"] = np.tile(np.arange(P, dtype=f32)[None, :], (P, 1))
    c["ident"] = np.eye(P, dtype=f32)
    c["ones"] = np.ones((P, 1), dtype=f32)
    c["dum_a"] = np.full((1, 576), NEGBIG, dtype=f32)
    c["dum_s"] = np.zeros((1, 64), dtype=f32)

    def fuse(W, a_s, a_d):
        # WW = [W | W@blockdiag(a_src) | W@blockdiag(a_dst)]  -> [Fin, HC+16]
        H, C = a_s.shape
        Ws = np.einsum("fhc,hc->fh", W.reshape(-1, H, C), a_s)
        Wd = np.einsum("fhc,hc->fh", W.reshape(-1, H, C), a_d)
        return np.concatenate([W, Ws, Wd], axis=1).astype(f32)

    c["ww1"] = fuse(inputs["We1"], inputs["as_e1"], inputs["ad_e1"])
    c["ww2"] = fuse(inputs["We2"], inputs["as_e2"], inputs["ad_e2"])
    c["ww3"] = fuse(inputs["Wd1"], inputs["as_d1"], inputs["ad_d1"])
    c["ww4"] = fuse(inputs["Wd2"], inputs["as_d2"], inputs["ad_d2"])
    # edge-output biases broadcast to 128 partitions
    c["bb1"] = np.tile(inputs["b_e1"][None, :], (P, 1)).astype(f32)  # [128,128]
    c["bb2"] = np.tile(inputs["b_e2"][None, :], (P, 1)).astype(f32)  # [128,32]
    c["bb3"] = np.tile(inputs["b_d1"][None, :], (P, 1)).astype(f32)  # [128,128]
    c["bb4"] = np.tile(inputs["b_d2"][None, :], (P, 1)).astype(f32)  # [128,64]
    c["bn1g"] = inputs["bn1_g"].astype(f32).reshape(-1, 1)  # [128,1]
    c["bn1b"] = inputs["bn1_b"].astype(f32).reshape(-1, 1)
    c["bn2g"] = inputs["bn2_g"].astype(f32).reshape(-1, 1)
    c["bn2b"] = inputs["bn2_b"].astype(f32).reshape(-1, 1)
    c["xin"] = np.asarray(inputs["x"], dtype=f32)
    return c


def _build(NCH, dbg=None, repeat_k=1):
    from concourse import bacc, bass, mybir, tile

    f32 = mybir.dt.float32
    i16 = mybir.dt.int16
    nc = bacc.Bacc(
        "TRN2",
        target_bir_lowering=False,
        debug=False,
        enable_asserts=False,
        num_devices=M,
    )

    EPT = NCH * P

    def din(name, shape, dtype=f32):
        return nc.dram_tensor(name, list(shape), dtype, kind="ExternalInput")

    xin = din("xin", (N, 64))
    idx_src = din("idx_src", (128, NT * EPT // 16), i16)
    idx_dst = din("idx_dst", (128, NT * EPT // 16), i16)
    dstloc = din("dstloc", (P, NT * NCH))
    iotaf = din("iotaf", (P, P))
    ident = din("ident", (P, P))
    ones = din("ones", (P, 1))
    dum_a = din("dum_a", (1, 576))
    dum_s = din("dum_s", (1, 64))
    ww = [din(f"ww{l + 1}", (LAYERS[l]["Fin"], HEADS * LAYERS[l]["C"] + 16)) for l in range(4)]
    bb = [
        din("bb1", (P, 128)),
        din("bb2", (P, 32)),
        din("bb3", (P, 128)),
        din("bb4", (P, 64)),
    ]
    bng = [None, din("bn1g", (128, 1)), None, din("bn2g", (128, 1))]
    bnb = [None, din("bn1b", (128, 1)), None, din("bn2b", (128, 1))]
    if dbg == "tab0":
        out_ext = nc.dram_tensor("out", [N, 144], f32, kind="ExternalOutput")
    elif dbg in ("own0", "xg0"):
        out_ext = nc.dram_tensor("out", [N if dbg == "xg0" else NPC, 128], f32, kind="ExternalOutput")
    elif dbg == "tab1":
        out_ext = nc.dram_tensor("out", [N, 272], f32, kind="ExternalOutput")
    elif dbg == "own1":
        out_ext = nc.dram_tensor("out", [NPC, 32], f32, kind="ExternalOutput")
    else:
        out_ext = nc.dram_tensor("out", [NPC, 64], f32, kind="ExternalOutput")

    with tile.TileContext(nc) as tc:
        with (
            tc.tile_pool(name="dram", bufs=1, space="DRAM") as dram,
            tc.tile_pool(name="const", bufs=1) as cpool,
            tc.tile_pool(name="work", bufs=3) as wpool,
            tc.tile_pool(name="gath", bufs=2) as gpool,
            tc.tile_pool(name="psum", bufs=2, space="PSUM") as ppool,
        ):
            # ---- internal DRAM ----
            tabA = [dram.tile([N + 1, LAYERS[l]["R"]], f32, tag=f"tabA{l}", name=f"tabA{l}") for l in range(4)]
            tabS = dram.tile([N + 1, 64], f32, tag="tabS", name="tabS")
            own = [
                dram.tile([NPC, 128], f32, tag="own0", name="own0"),
                dram.tile([NPC, 32], f32, tag="own1", name="own1"),
                dram.tile([NPC, 128], f32, tag="own2", name="own2"),
            ]
            xg = [
                dram.tile([N, 128], f32, tag="xg0", name="xg0", addr_space="Shared"),
                dram.tile([N, 32], f32, tag="xg1", name="xg1", addr_space="Shared"),
                dram.tile([N, 128], f32, tag="xg2", name="xg2", addr_space="Shared"),
            ]

            # ---- consts to SBUF ----
            def load_const(ap, shape, dtype=f32, tag=None):
                t = cpool.tile(list(shape), dtype, tag=tag, name=tag)
                nc.sync.dma_start(out=t[:], in_=ap[:])
                return t

            iotaf_sb = load_const(iotaf, (P, P), tag="iotaf")
            ident_sb = load_const(ident, (P, P), tag="ident")
            ones_sb = load_const(ones, (P, 1), tag="ones")
            isrc_sb = load_const(idx_src, (128, NT * EPT // 16), i16, tag="isrc")
            idst_sb = load_const(idx_dst, (128, NT * EPT // 16), i16, tag="idst")
            dstloc_sb = load_const(dstloc, (P, NT * NCH), tag="dstloc")
            ww_sb = [
                load_const(ww[l], (LAYERS[l]["Fin"], HEADS * LAYERS[l]["C"] + 16), tag=f"ww{l}")
                for l in range(4)
            ]
            bb_sb = [
                load_const(bb[0], (P, 128), tag="bb0"),
                load_const(bb[1], (P, 32), tag="bb1"),
                load_const(bb[2], (P, 128), tag="bb2"),
                load_const(bb[3], (P, 64), tag="bb3"),
            ]
            bng_sb = [None, load_const(bng[1], (128, 1), tag="bng1"), None, load_const(bng[3], (128, 1), tag="bng3")]
            bnb_sb = [None, load_const(bnb[1], (128, 1), tag="bnb1"), None, load_const(bnb[3], (128, 1), tag="bnb3")]

            # dummy rows for pad edges
            for l in range(4):
                nc.sync.dma_start(
                    out=tabA[l][N : N + 1, : LAYERS[l]["R"]], in_=dum_a[:1, : LAYERS[l]["R"]]
                )
            nc.sync.dma_start(out=tabS[N : N + 1, :], in_=dum_s[:1, :])

            AX = mybir.AxisListType.X
            OP = mybir.AluOpType
            AF = mybir.ActivationFunctionType

            # register holding num_idxs for dma_gather (Tile-safe dynamic value)
            _greg = nc.alloc_registers(name="eptreg")
            nc.regs_mov(_greg, EPT)
            ept_reg = nc.snap(_greg, donate=False)

            # ================= node phase =================
            def node_phase(l, src_dram):
                cfg = LAYERS[l]
                Fin, C, R = cfg["Fin"], cfg["C"], cfg["R"]
                HC = HEADS * C
                scale_off = None
                if cfg["bn"]:
                    psS = ppool.tile([Fin, 1], f32, tag="psS", name="psS", bufs=1)
                    psS2 = ppool.tile([Fin, 1], f32, tag="psS2", name="psS2", bufs=1)
                    for i in range(NODE_TILES):
                        cnt = min(P, N - i * P)
                        xt = wpool.tile([P, Fin], f32, tag="xt_st", name="xt_st")
                        if cnt < P:
                            nc.vector.memset(xt[:], 0.0)
                        nc.sync.dma_start(out=xt[:cnt, :], in_=src_dram[i * P : i * P + cnt, :])
                        sq = wpool.tile([P, Fin], f32, tag="sq_st", name="sq_st")
                        nc.scalar.square(sq[:], xt[:])
                        nc.tensor.matmul(
                            out=psS[:], lhsT=xt[:], rhs=ones_sb[:],
                            start=(i == 0), stop=(i == NODE_TILES - 1),
                        )
                        nc.tensor.matmul(
                            out=psS2[:], lhsT=sq[:], rhs=ones_sb[:],
                            start=(i == 0), stop=(i == NODE_TILES - 1),
                        )
                    mu = wpool.tile([Fin, 1], f32, tag="mu", name="mu")
                    nc.vector.tensor_scalar(mu[:], psS[:], 1.0 / N, None, OP.mult)
                    msq = wpool.tile([Fin, 1], f32, tag="msq", name="msq")
                    nc.vector.tensor_scalar(msq[:], psS2[:], 1.0 / N, None, OP.mult)
                    var = wpool.tile([Fin, 1], f32, tag="var", name="var")
                    nc.vector.tensor_tensor(var[:], mu[:], mu[:], OP.mult)
                    nc.vector.tensor_tensor(var[:], msq[:], var[:], OP.subtract)
                    nc.vector.tensor_scalar(var[:], var[:], BN_EPS, None, OP.add)
                    sdv = wpool.tile([Fin, 1], f32, tag="sdv", name="sdv")
                    nc.scalar.activation(sdv[:], var[:], AF.Sqrt)
                    rs = wpool.tile([Fin, 1], f32, tag="rs", name="rs")
                    nc.vector.reciprocal(rs[:], sdv[:])
                    bscale = wpool.tile([Fin, 1], f32, tag="bscale", name="bscale")
                    nc.vector.tensor_tensor(bscale[:], rs[:], bng_sb[l][:], OP.mult)
                    boff = wpool.tile([Fin, 1], f32, tag="boff", name="boff")
                    nc.vector.tensor_tensor(boff[:], mu[:], bscale[:], OP.mult)
                    nc.vector.tensor_tensor(boff[:], bnb_sb[l][:], boff[:], OP.subtract)
                    scale_off = (bscale, boff)

                for i in range(NODE_TILES):
                    cnt = min(P, N - i * P)
                    xt = wpool.tile([P, Fin], f32, tag="xt", name="xt")
                    if cnt < P:
                        nc.vector.memset(xt[:], 0.0)
                    nc.sync.dma_start(out=xt[:cnt, :], in_=src_dram[i * P : i * P + cnt, :])
                    xtp = ppool.tile([Fin, P], f32, tag="xtp", name="xtp", bufs=2)
                    nc.tensor.transpose(out=xtp[:], in_=xt[:], identity=ident_sb[:])
                    xts = wpool.tile([Fin, P], f32, tag="xts", name="xts")
                    if scale_off is not None:
                        nc.vector.tensor_scalar(
                            xts[:], xtp[:], scale_off[0][:], scale_off[1][:], OP.mult, OP.add
                        )
                        nc.scalar.activation(xts[:], xts[:], AF.Relu)
                    else:
                        nc.vector.tensor_copy(xts[:], xtp[:])
                    if HC + 16 <= 512:
                        hp = ppool.tile([P, HC + 16], f32, tag="pmm", name="hp", bufs=2)
                        nc.tensor.matmul(out=hp[:], lhsT=xts[:], rhs=ww_sb[l][:], start=True, stop=True)
                        hpA, hpB_s, hpB_d = hp[:, : HC + 8], hp[:, HC + 8 : HC + 16], None
                    else:  # L4: 528 cols -> split 512 + 16
                        hp = ppool.tile([P, 512], f32, tag="pmm", name="hp", bufs=2)
                        hp2 = ppool.tile([P, 16], f32, tag="pmm2", name="hp2", bufs=2)
                        nc.tensor.matmul(out=hp[:], lhsT=xts[:], rhs=ww_sb[l][:, :512], start=True, stop=True)
                        nc.tensor.matmul(out=hp2[:], lhsT=xts[:], rhs=ww_sb[l][:, 512:], start=True, stop=True)
                        hpA, hpB_s, hpB_d = None, hp2[:, 0:8], hp2[:, 8:16]
                    tt = wpool.tile([P, HC + 8], f32, tag="tt", name="tt")
                    if hpA is not None:
                        nc.vector.tensor_copy(tt[:], hpA)
                        sd = wpool.tile([P, 8], f32, tag="sd", name="sd")
                        nc.vector.tensor_copy(sd[:], hpB_s)
                    else:
                        nc.vector.tensor_copy(tt[:, :512], hp[:])
                        nc.vector.tensor_copy(tt[:, 512:520], hpB_s)
                        sd = wpool.tile([P, 8], f32, tag="sd", name="sd")
                        nc.vector.tensor_copy(sd[:], hpB_d)
                    nc.sync.dma_start(
                        out=tabA[l][i * P : i * P + cnt, : HC + 8], in_=tt[:cnt, :]
                    )
                    nc.sync.dma_start(out=tabS[i * P : i * P + cnt, :8], in_=sd[:cnt, :])

            # ================= edge phase =================
            def edge_phase(l, out_dram):
                cfg = LAYERS[l]
                C, R = cfg["C"], cfg["R"]
                HC = HEADS * C
                for t in range(NT):
                    cnt = min(P, NPC - t * P)
                    G = gpool.tile([P, NCH * R], f32, tag="G", name="G")
                    nc.gpsimd.dma_gather(
                        out_ap=G[:].rearrange("p (j r) -> p j r", r=R),
                        in_ap=tabA[l][:],
                        idxs_ap=isrc_sb[:, t * EPT // 16 : (t + 1) * EPT // 16],
                        num_idxs=EPT,
                        num_idxs_reg=ept_reg,
                        elem_size=R,
                        single_packet=False,
                    )
                    SD = gpool.tile([P, NCH * 64], f32, tag="SD", name="SD")
                    nc.gpsimd.dma_gather(
                        out_ap=SD[:].rearrange("p (j r) -> p j r", r=64),
                        in_ap=tabS[:],
                        idxs_ap=idst_sb[:, t * EPT // 16 : (t + 1) * EPT // 16],
                        num_idxs=EPT,
                        num_idxs_reg=ept_reg,
                        elem_size=64,
                        single_packet=False,
                    )
                    if HC + 8 <= 512:
                        psA = ppool.tile([P, HC + 8], f32, tag="pmm", name="psA", bufs=2)
                        psB = None
                    else:
                        psA = ppool.tile([P, 512], f32, tag="pmm", name="psA", bufs=2)
                        psB = ppool.tile([P, 8], f32, tag="pmm2", name="psB", bufs=2)
                    for j in range(NCH):
                        Gj = G[:, j * R : (j + 1) * R]
                        ebuf = wpool.tile([P, 8], f32, tag="ebuf", name="ebuf")
                        nc.vector.tensor_tensor(
                            ebuf[:], Gj[:, HC : HC + 8], SD[:, j * 64 : j * 64 + 8], OP.add
                        )
                        eb2 = wpool.tile([P, 8], f32, tag="eb2", name="eb2")
                        nc.vector.tensor_scalar(eb2[:], ebuf[:], NEG, None, OP.mult)
                        nc.vector.tensor_tensor(ebuf[:], ebuf[:], eb2[:], OP.max)
                        GEX = wpool.tile([P, HC + 8], f32, tag="GEX", name="GEX")
                        nc.scalar.activation(GEX[:, HC : HC + 8], ebuf[:], AF.Exp)
                        O = wpool.tile([P, P], f32, tag="O", name="O")
                        col = t * NCH + j
                        nc.vector.tensor_scalar(
                            O[:], iotaf_sb[:], dstloc_sb[:, col : col + 1], None, OP.is_equal
                        )
                        nc.vector.tensor_tensor(
                            GEX[:, :HC].rearrange("p (h c) -> p h c", h=HEADS),
                            Gj[:, :HC].rearrange("p (h c) -> p h c", h=HEADS),
                            GEX[:, HC : HC + 8].unsqueeze(2).to_broadcast((P, HEADS, C)),
                            OP.mult,
                        )
                        if psB is None:
                            nc.tensor.matmul(
                                out=psA[:], lhsT=O[:], rhs=GEX[:],
                                start=(j == 0), stop=(j == NCH - 1),
                            )
                        else:
                            nc.tensor.matmul(
                                out=psA[:], lhsT=O[:], rhs=GEX[:, :512],
                                start=(j == 0), stop=(j == NCH - 1),
                            )
                            nc.tensor.matmul(
                                out=psB[:], lhsT=O[:], rhs=GEX[:, 512:520],
                                start=(j == 0), stop=(j == NCH - 1),
                            )
                    den = psA[:, HC : HC + 8] if psB is None else psB[:]
                    rec = wpool.tile([P, 8], f32, tag="rec", name="rec")
                    nc.vector.tensor_scalar(rec[:], den, 1e-16, None, OP.add)
                    nc.vector.reciprocal(rec[:], rec[:])
                    res = wpool.tile([P, HC], f32, tag="res", name="res")
                    nc.vector.tensor_tensor(
                        res[:].rearrange("p (h c) -> p h c", h=HEADS),
                        psA[:, :HC].rearrange("p (h c) -> p h c", h=HEADS),
                        rec[:].unsqueeze(2).to_broadcast((P, HEADS, C)),
                        OP.mult,
                    )
                    if cfg["concat"]:
                        nc.vector.tensor_tensor(res[:], res[:], bb_sb[l][:], OP.add)
                        nc.sync.dma_start(
                            out=out_dram[t * P : t * P + cnt, :], in_=res[:cnt, :]
                        )
                    else:
                        red = wpool.tile([P, C], f32, tag="red", name="red")
                        nc.vector.tensor_reduce(
                            red[:],
                            res[:].rearrange("p (h c) -> p c h", h=HEADS),
                            AX,
                            OP.add,
                        )
                        nc.vector.tensor_scalar(red[:], red[:], 1.0 / HEADS, None, OP.mult)
                        nc.vector.tensor_tensor(red[:], red[:], bb_sb[l][:, :C], OP.add)
                        nc.sync.dma_start(
                            out=out_dram[t * P : t * P + cnt, :], in_=red[:cnt, :]
                        )

            # ================= full pipeline =================
            srcs = [xin, xg[0], xg[1], xg[2]]
            outs = [own[0], own[1], own[2], out_ext]
            def agather(l):
                nc.gpsimd.collective_compute(
                    "AllGather",
                    mybir.AluOpType.bypass,
                    replica_groups=[list(range(M))],
                    ins=[own[l].opt()],
                    outs=[xg[l].opt()],
                )
            if dbg == "tab0":
                node_phase(0, xin)
                hc8 = 136
                for i in range(NODE_TILES):
                    cnt = min(P, N - i * P)
                    tbuf = wpool.tile([P, 144], f32, tag="tbuf", name="tbuf")
                    nc.sync.dma_start(out=tbuf[:cnt, :hc8], in_=tabA[0][i*P:i*P+cnt, :hc8])
                    nc.sync.dma_start(out=tbuf[:cnt, hc8:144], in_=tabS[i*P:i*P+cnt, :8])
                    nc.sync.dma_start(out=out_ext[i*P:i*P+cnt, :], in_=tbuf[:cnt, :])
            elif dbg == "own0":
                node_phase(0, xin)
                edge_phase(0, out_ext)
            elif dbg == "xg0":
                node_phase(0, xin)
                edge_phase(0, own[0])
                agather(0)
                for i in range(NODE_TILES):
                    cnt = min(P, N - i * P)
                    tbuf = wpool.tile([P, 128], f32, tag="tbuf", name="tbuf")
                    nc.sync.dma_start(out=tbuf[:cnt, :], in_=xg[0][i*P:i*P+cnt, :])
                    nc.sync.dma_start(out=out_ext[i*P:i*P+cnt, :], in_=tbuf[:cnt, :])
            elif dbg == "tab1":
                node_phase(0, xin); edge_phase(0, own[0]); agather(0)
                node_phase(1, xg[0])
                hc8 = 264
                for i in range(NODE_TILES):
                    cnt = min(P, N - i * P)
                    tbuf = wpool.tile([P, 272], f32, tag="tbuf", name="tbuf")
                    nc.sync.dma_start(out=tbuf[:cnt, :hc8], in_=tabA[1][i*P:i*P+cnt, :hc8])
                    nc.sync.dma_start(out=tbuf[:cnt, hc8:272], in_=tabS[i*P:i*P+cnt, :8])
                    nc.sync.dma_start(out=out_ext[i*P:i*P+cnt, :], in_=tbuf[:cnt, :])
            elif dbg == "own1":
                node_phase(0, xin); edge_phase(0, own[0]); agather(0)
                node_phase(1, xg[0]); edge_phase(1, out_ext)
            else:
                for _rep in range(repeat_k):
                    if _rep > 0:
                        # Shared tensors allow a single writing instruction;
                        # later reps gather into their own buffers.
                        for l, w in [(0, 128), (1, 32), (2, 128)]:
                            xg[l] = dram.tile([N, w], f32, tag=f"xg{l}r{_rep}",
                                              name=f"xg{l}r{_rep}", addr_space="Shared")
                        srcs = [xin, xg[0], xg[1], xg[2]]
                    for l in range(4):
                        node_phase(l, srcs[l])
                        edge_phase(l, outs[l])
                        if l < 3:
                            agather(l)
    if not nc.is_finalized():
        nc.finalize()
    return nc


def _pjrt_exec(nc, in_maps, time_reps=0):
    """Mirror of bass2jax.run_bass_via_pjrt multi-core path, holding the jitted
    executable so repeated executions can be wall-timed (NTFF profiling is
    unavailable in this container)."""
    import time as _t
    import jax
    from jax.experimental.shard_map import shard_map
    from jax.sharding import Mesh, PartitionSpec
    from concourse import bass2jax as B, mybir as mb

    B.install_neuronx_cc_hook()
    n_cores = len(in_maps)
    partition_name = nc.partition_id_tensor.name if nc.partition_id_tensor else None
    in_names, out_names, out_avals, zero_outs = [], [], [], []
    for alloc in nc.m.functions[0].allocations:
        if not isinstance(alloc, mb.MemoryLocationSet):
            continue
        name = alloc.memorylocations[0].name
        if alloc.kind == "ExternalInput":
            if name != partition_name:
                in_names.append(name)
        elif alloc.kind == "ExternalOutput":
            out_names.append(name)
            shape = tuple(alloc.tensor_shape)
            dtype = mb.dt.np(alloc.dtype)
            out_avals.append(jax.core.ShapedArray(shape, dtype))
            zero_outs.append(np.zeros(shape, dtype))
    n_params = len(in_names)
    n_outs = len(out_avals)
    in_names.extend(out_names)
    if partition_name is not None:
        in_names.append(partition_name)
    donate = tuple(range(n_params, n_params + n_outs))

    def _body(*args):
        operands = list(args)
        if partition_name is not None:
            operands.append(B.partition_id_tensor())
        outs = B._bass_exec_p.bind(
            *operands,
            out_avals=tuple(out_avals),
            in_names=tuple(in_names),
            out_names=tuple(out_names),
            lowering_input_output_aliases=(),
            sim_require_finite=True,
            sim_require_nnan=True,
            nc=nc,
        )
        return tuple(outs)

    devices = jax.devices()[:n_cores]
    mesh = Mesh(np.asarray(devices), ("core",))
    in_specs = (PartitionSpec("core"),) * (n_params + n_outs)
    out_specs = (PartitionSpec("core"),) * len(out_names)
    sharded = jax.jit(
        shard_map(_body, mesh=mesh, in_specs=in_specs, out_specs=out_specs,
                  check_rep=False),
        donate_argnums=donate, keep_unused=True,
    )
    per_core = [[np.asarray(m_[nm]) for nm in in_names[:n_params]] for m_ in in_maps]
    concat_in = [
        np.concatenate([per_core[c][i] for c in range(n_cores)], axis=0)
        for i in range(n_params)
    ]
    from jax.sharding import NamedSharding
    shard = NamedSharding(mesh, PartitionSpec("core"))
    concat_in = [jax.device_put(a, shard) for a in concat_in]
    jax.block_until_ready(concat_in)

    def once():
        cz = [jax.device_put(np.zeros((n_cores * z.shape[0], *z.shape[1:]), z.dtype), shard)
              for z in zero_outs]
        jax.block_until_ready(cz)
        t0 = _t.perf_counter()
        out_arrs = sharded(*concat_in, *cz)
        jax.block_until_ready(out_arrs)
        return _t.perf_counter() - t0, out_arrs

    _, out_arrs = once()  # compile + first run
    times = []
    for _ in range(time_reps):
        dt, out_arrs = once()
        times.append(dt)
    res = [
        {nm: np.asarray(out_arrs[i]).reshape(n_cores, *out_avals[i].shape)[c]
         for i, nm in enumerate(out_names)}
        for c in range(n_cores)
    ]
    return res, (min(times) if times else None)


def _run(inputs, trace=False, time_reps=0, dbg=None, repeat_k=1):
    NCH, edata = _preprocess(np.asarray(inputs["edge_index"]))
    consts = _host_consts(inputs)
    nc = _build(NCH, dbg=dbg, repeat_k=repeat_k)

    in_maps = []
    for m in range(M):
        d = dict(consts)
        d.update(edata[m])
        in_maps.append(d)

    if time_reps > 0:
        results, best_s = _pjrt_exec(nc, in_maps, time_reps=time_reps)
    else:
        from concourse.bass_utils import run_bass_kernel_spmd

        res = run_bass_kernel_spmd(nc, in_maps, core_ids=list(range(M)))
        results, best_s = res.results, None
    outs = [np.asarray(results[m]["out"]) for m in range(M)]
    full = np.concatenate(outs, axis=0).astype(np.float32)
    return full, (None if best_s is None else int(best_s * 1e9))


def kernel(**inputs):
    out, _ = _run(inputs, trace=False)
    return out



# revision 5
# speedup vs baseline: 11.7519x; 11.7519x over previous
"""Distributed Bass kernel for a 4-layer GAT autoencoder on 8 TRN2 NeuronCores.

Strategy (per sharding hint): nodes sharded across 8 cores (2500/core);
edges co-located with their destination node's core, sorted by destination;
params replicated. Node-level compute (x@W, attention score projections) is
replicated on every core (cheap); the edge phase (gather + segment softmax +
weighted scatter) is sharded by destination. Layer outputs are AllGathered
in-kernel so the next layer's node phase sees the full node set.

Edge phase per 128-edge chunk:
  - dma_gather source-node rows [h | s_src] from a DRAM table (by src id)
  - dma_gather destination rows [s_dst] (by dst id)
  - e = LeakyReLU(s_src + s_dst); ex = exp(e)  (softmax max-shift dropped:
    exp ratios are shift-invariant and |e| is small for this data)
  - one-hot matrix O[e,d] = (dstloc[e]==d) built via is_equal vs an iota tile
  - PSUM += O.T @ [h*ex | ex]  accumulates both numerator and denominator
  - after all chunks: out = num/den (+1e-16), bias, head-concat or head-mean
"""

import sys

sys.path.insert(0, "/opt/trn_rl_repo")

import numpy as np

P = 128
M = 8
N = 20000
NPC = N // M  # 2500 nodes per core
NT = (NPC + P - 1) // P  # 20 dst tiles per core
HEADS = 8
NEG = 0.2
BN_EPS = 1e-5
DUMMY = N  # dummy table row for pad edges
NEGBIG = -1.0e30

# layer configs: Fin, C (per-head out), concat?, bn on input?, table row width R
LAYERS = [
    dict(Fin=64, C=16, concat=True, bn=False, R=192),
    dict(Fin=128, C=32, concat=False, bn=True, R=320),
    dict(Fin=32, C=16, concat=True, bn=False, R=192),
    dict(Fin=128, C=64, concat=False, bn=True, R=576),
]
NODE_TILES = (N + P - 1) // P  # 157 (last has 32 rows)


def _wrap16(idx):
    """Host int array -> dma_gather index layout [16, n/16] (idx[s*16+p] at [p,s])."""
    n = idx.shape[0]
    assert n % 16 == 0
    w = np.ascontiguousarray(idx.reshape(n // 16, 16).T).astype(np.int16)
    return np.ascontiguousarray(np.tile(w, (8, 1)))  # replicated for the 8 Q7 cores


def _preprocess(edge_index):
    """Partition + sort edges; build per-core gather/index arrays."""
    src = np.concatenate([np.asarray(edge_index[0]), np.arange(N)]).astype(np.int64)
    dst = np.concatenate([np.asarray(edge_index[1]), np.arange(N)]).astype(np.int64)

    per_core = []
    max_cnt = 0
    for m in range(M):
        sel = (dst // NPC) == m
        s, d = src[sel], dst[sel]
        dloc = d - NPC * m
        order = np.argsort(dloc, kind="stable")
        s, dloc = s[order], dloc[order]
        tiles = []
        for t in range(NT):
            tsel = (dloc // P) == t
            st, dt_ = s[tsel], dloc[tsel] - t * P
            tiles.append((st, dt_))
            max_cnt = max(max_cnt, st.shape[0])
        per_core.append(tiles)

    NCH = (max_cnt + P - 1) // P  # chunks per dst tile (uniform across cores)
    NCH = ((NCH + 3) // 4) * 4  # idx segments 64B-aligned for dma_gather ucode
    EPT = NCH * P  # padded edges per dst tile

    data = []
    for m in range(M):
        isrc = np.full((NT, EPT), DUMMY, dtype=np.int64)
        idst = np.full((NT, EPT), DUMMY, dtype=np.int64)
        dloc = np.full((NT, EPT), P - 1, dtype=np.int64)
        for t, (st, dt_) in enumerate(per_core[m]):
            c = st.shape[0]
            isrc[t, :c] = st
            idst[t, :c] = NPC * m + t * P + dt_
            dloc[t, :c] = dt_
        # dstloc layout [128, NT*NCH]: column t*NCH+j, row p = edge (t, j*128+p)
        dstloc = (
            dloc.reshape(NT, NCH, P).transpose(2, 0, 1).reshape(P, NT * NCH)
        ).astype(np.float32)
        data.append(
            dict(
                idx_src=_wrap16(isrc.reshape(-1)),
                idx_dst=_wrap16(idst.reshape(-1)),
                dstloc=np.ascontiguousarray(dstloc),
            )
        )
    return NCH, data


def _host_consts(inputs):
    """Fused weights + broadcast biases + misc consts (all replicated)."""
    f32 = np.float32
    c = {}
    c["iotaf"] = np.tile(np.arange(P, dtype=f32)[None, :], (P, 1))
    c["ident"] = np.eye(P, dtype=f32)
    c["ones"] = np.ones((P, 1), dtype=f32)
    c["dum_a"] = np.full((1, 576), NEGBIG, dtype=f32)
    c["dum_s"] = np.zeros((1, 64), dtype=f32)

    def fuse(W, a_s, a_d):
        # WW = [W | W@blockdiag(a_src) | W@blockdiag(a_dst)]  -> [Fin, HC+16]
        H, C = a_s.shape
        Ws = np.einsum("fhc,hc->fh", W.reshape(-1, H, C), a_s)
        Wd = np.einsum("fhc,hc->fh", W.reshape(-1, H, C), a_d)
        return np.concatenate([W, Ws, Wd], axis=1).astype(f32)

    c["ww1"] = fuse(inputs["We1"], inputs["as_e1"], inputs["ad_e1"])
    c["ww2"] = fuse(inputs["We2"], inputs["as_e2"], inputs["ad_e2"])
    c["ww3"] = fuse(inputs["Wd1"], inputs["as_d1"], inputs["ad_d1"])
    c["ww4"] = fuse(inputs["Wd2"], inputs["as_d2"], inputs["ad_d2"])
    # edge-output biases broadcast to 128 partitions
    c["bb1"] = np.tile(inputs["b_e1"][None, :], (P, 1)).astype(f32)  # [128,128]
    c["bb2"] = np.tile(inputs["b_e2"][None, :], (P, 1)).astype(f32)  # [128,32]
    c["bb3"] = np.tile(inputs["b_d1"][None, :], (P, 1)).astype(f32)  # [128,128]
    c["bb4"] = np.tile(inputs["b_d2"][None, :], (P, 1)).astype(f32)  # [128,64]
    c["bn1g"] = inputs["bn1_g"].astype(f32).reshape(-1, 1)  # [128,1]
    c["bn1b"] = inputs["bn1_b"].astype(f32).reshape(-1, 1)
    c["bn2g"] = inputs["bn2_g"].astype(f32).reshape(-1, 1)
    c["bn2b"] = inputs["bn2_b"].astype(f32).reshape(-1, 1)
    c["xin"] = np.asarray(inputs["x"], dtype=f32)
    return c


def _build(NCH, dbg=None, repeat_k=1, sim1=False):
    from concourse import bacc, bass, mybir, tile

    f32 = mybir.dt.float32
    i16 = mybir.dt.int16
    nc = bacc.Bacc(
        "TRN2",
        target_bir_lowering=False,
        debug=False,
        enable_asserts=False,
        num_devices=1 if sim1 else M,
    )

    EPT = NCH * P

    def din(name, shape, dtype=f32):
        return nc.dram_tensor(name, list(shape), dtype, kind="ExternalInput")

    xin = din("xin", (N, 64))
    idx_src = din("idx_src", (128, NT * EPT // 16), i16)
    idx_dst = din("idx_dst", (128, NT * EPT // 16), i16)
    dstloc = din("dstloc", (P, NT * NCH))
    iotaf = din("iotaf", (P, P))
    ident = din("ident", (P, P))
    ones = din("ones", (P, 1))
    dum_a = din("dum_a", (1, 576))
    dum_s = din("dum_s", (1, 64))
    ww = [din(f"ww{l + 1}", (LAYERS[l]["Fin"], HEADS * LAYERS[l]["C"] + 16)) for l in range(4)]
    bb = [
        din("bb1", (P, 128)),
        din("bb2", (P, 32)),
        din("bb3", (P, 128)),
        din("bb4", (P, 64)),
    ]
    bng = [None, din("bn1g", (128, 1)), None, din("bn2g", (128, 1))]
    bnb = [None, din("bn1b", (128, 1)), None, din("bn2b", (128, 1))]
    if dbg == "tab0":
        out_ext = nc.dram_tensor("out", [N, 144], f32, kind="ExternalOutput")
    elif dbg in ("own0", "xg0"):
        out_ext = nc.dram_tensor("out", [N if dbg == "xg0" else NPC, 128], f32, kind="ExternalOutput")
    elif dbg == "tab1":
        out_ext = nc.dram_tensor("out", [N, 272], f32, kind="ExternalOutput")
    elif dbg == "own1":
        out_ext = nc.dram_tensor("out", [NPC, 32], f32, kind="ExternalOutput")
    else:
        out_ext = nc.dram_tensor("out", [NPC, 64], f32, kind="ExternalOutput")

    with tile.TileContext(nc) as tc:
        with (
            tc.tile_pool(name="dram", bufs=1, space="DRAM") as dram,
            tc.tile_pool(name="const", bufs=1) as cpool,
            tc.tile_pool(name="work", bufs=3) as wpool,
            tc.tile_pool(name="gath", bufs=2) as gpool,
            tc.tile_pool(name="psum", bufs=2, space="PSUM") as ppool,
        ):
            # ---- internal DRAM ----
            tabA = [dram.tile([N + 1, LAYERS[l]["R"]], f32, tag=f"tabA{l}", name=f"tabA{l}") for l in range(4)]
            tabS = dram.tile([N + 1, 64], f32, tag="tabS", name="tabS")
            own = [
                dram.tile([NPC, 128], f32, tag="own0", name="own0"),
                dram.tile([NPC, 32], f32, tag="own1", name="own1"),
                dram.tile([NPC, 128], f32, tag="own2", name="own2"),
            ]
            _xg_kw = {} if sim1 else dict(addr_space="Shared")
            xg = [
                dram.tile([N, 128], f32, tag="xg0", name="xg0", **_xg_kw),
                dram.tile([N, 32], f32, tag="xg1", name="xg1", **_xg_kw),
                dram.tile([N, 128], f32, tag="xg2", name="xg2", **_xg_kw),
            ]

            # ---- consts to SBUF ----
            def load_const(ap, shape, dtype=f32, tag=None):
                t = cpool.tile(list(shape), dtype, tag=tag, name=tag)
                nc.sync.dma_start(out=t[:], in_=ap[:])
                return t

            iotaf_sb = load_const(iotaf, (P, P), tag="iotaf")
            ident_sb = load_const(ident, (P, P), tag="ident")
            ones_sb = load_const(ones, (P, 1), tag="ones")
            isrc_sb = load_const(idx_src, (128, NT * EPT // 16), i16, tag="isrc")
            idst_sb = load_const(idx_dst, (128, NT * EPT // 16), i16, tag="idst")
            dstloc_sb = load_const(dstloc, (P, NT * NCH), tag="dstloc")
            ww_sb = [
                load_const(ww[l], (LAYERS[l]["Fin"], HEADS * LAYERS[l]["C"] + 16), tag=f"ww{l}")
                for l in range(4)
            ]
            bb_sb = [
                load_const(bb[0], (P, 128), tag="bb0"),
                load_const(bb[1], (P, 32), tag="bb1"),
                load_const(bb[2], (P, 128), tag="bb2"),
                load_const(bb[3], (P, 64), tag="bb3"),
            ]
            bng_sb = [None, load_const(bng[1], (128, 1), tag="bng1"), None, load_const(bng[3], (128, 1), tag="bng3")]
            bnb_sb = [None, load_const(bnb[1], (128, 1), tag="bnb1"), None, load_const(bnb[3], (128, 1), tag="bnb3")]

            # dummy rows for pad edges
            for l in range(4):
                nc.sync.dma_start(
                    out=tabA[l][N : N + 1, : LAYERS[l]["R"]], in_=dum_a[:1, : LAYERS[l]["R"]]
                )
            nc.sync.dma_start(out=tabS[N : N + 1, :], in_=dum_s[:1, :])

            AX = mybir.AxisListType.X
            OP = mybir.AluOpType
            AF = mybir.ActivationFunctionType

            # register holding num_idxs for dma_gather (Tile-safe dynamic value)
            _greg = nc.alloc_registers(name="eptreg")
            nc.regs_mov(_greg, EPT)
            ept_reg = nc.snap(_greg, donate=False)

            # ================= node phase =================
            def node_phase(l, src_dram):
                cfg = LAYERS[l]
                Fin, C, R = cfg["Fin"], cfg["C"], cfg["R"]
                HC = HEADS * C
                scale_off = None
                if cfg["bn"]:
                    psS = ppool.tile([Fin, 1], f32, tag="psS", name="psS", bufs=1)
                    psS2 = ppool.tile([Fin, 1], f32, tag="psS2", name="psS2", bufs=1)
                    for i in range(NODE_TILES):
                        cnt = min(P, N - i * P)
                        xt = wpool.tile([P, Fin], f32, tag="xt_st", name="xt_st")
                        if cnt < P:
                            nc.vector.memset(xt[:], 0.0)
                        nc.sync.dma_start(out=xt[:cnt, :], in_=src_dram[i * P : i * P + cnt, :])
                        sq = wpool.tile([P, Fin], f32, tag="sq_st", name="sq_st")
                        nc.scalar.square(sq[:], xt[:])
                        nc.tensor.matmul(
                            out=psS[:], lhsT=xt[:], rhs=ones_sb[:],
                            start=(i == 0), stop=(i == NODE_TILES - 1),
                        )
                        nc.tensor.matmul(
                            out=psS2[:], lhsT=sq[:], rhs=ones_sb[:],
                            start=(i == 0), stop=(i == NODE_TILES - 1),
                        )
                    mu = wpool.tile([Fin, 1], f32, tag="mu", name="mu")
                    nc.vector.tensor_scalar(mu[:], psS[:], 1.0 / N, None, OP.mult)
                    msq = wpool.tile([Fin, 1], f32, tag="msq", name="msq")
                    nc.vector.tensor_scalar(msq[:], psS2[:], 1.0 / N, None, OP.mult)
                    var = wpool.tile([Fin, 1], f32, tag="var", name="var")
                    nc.vector.tensor_tensor(var[:], mu[:], mu[:], OP.mult)
                    nc.vector.tensor_tensor(var[:], msq[:], var[:], OP.subtract)
                    nc.vector.tensor_scalar(var[:], var[:], BN_EPS, None, OP.add)
                    sdv = wpool.tile([Fin, 1], f32, tag="sdv", name="sdv")
                    nc.scalar.activation(sdv[:], var[:], AF.Sqrt)
                    rs = wpool.tile([Fin, 1], f32, tag="rs", name="rs")
                    nc.vector.reciprocal(rs[:], sdv[:])
                    bscale = wpool.tile([Fin, 1], f32, tag="bscale", name="bscale")
                    nc.vector.tensor_tensor(bscale[:], rs[:], bng_sb[l][:], OP.mult)
                    boff = wpool.tile([Fin, 1], f32, tag="boff", name="boff")
                    nc.vector.tensor_tensor(boff[:], mu[:], bscale[:], OP.mult)
                    nc.vector.tensor_tensor(boff[:], bnb_sb[l][:], boff[:], OP.subtract)
                    scale_off = (bscale, boff)

                for i in range(NODE_TILES):
                    cnt = min(P, N - i * P)
                    xt = wpool.tile([P, Fin], f32, tag="xt", name="xt")
                    if cnt < P:
                        nc.vector.memset(xt[:], 0.0)
                    nc.sync.dma_start(out=xt[:cnt, :], in_=src_dram[i * P : i * P + cnt, :])
                    xtp = ppool.tile([Fin, P], f32, tag="xtp", name="xtp", bufs=2)
                    nc.tensor.transpose(out=xtp[:], in_=xt[:], identity=ident_sb[:])
                    xts = wpool.tile([Fin, P], f32, tag="xts", name="xts")
                    if scale_off is not None:
                        nc.vector.tensor_scalar(
                            xts[:], xtp[:], scale_off[0][:], scale_off[1][:], OP.mult, OP.add
                        )
                        nc.scalar.activation(xts[:], xts[:], AF.Relu)
                    else:
                        nc.vector.tensor_copy(xts[:], xtp[:])
                    if HC + 16 <= 512:
                        hp = ppool.tile([P, HC + 16], f32, tag="pmm", name="hp", bufs=2)
                        nc.tensor.matmul(out=hp[:], lhsT=xts[:], rhs=ww_sb[l][:], start=True, stop=True)
                        hpA, hpB_s, hpB_d = hp[:, : HC + 8], hp[:, HC + 8 : HC + 16], None
                    else:  # L4: 528 cols -> split 512 + 16
                        hp = ppool.tile([P, 512], f32, tag="pmm", name="hp", bufs=2)
                        hp2 = ppool.tile([P, 16], f32, tag="pmm2", name="hp2", bufs=2)
                        nc.tensor.matmul(out=hp[:], lhsT=xts[:], rhs=ww_sb[l][:, :512], start=True, stop=True)
                        nc.tensor.matmul(out=hp2[:], lhsT=xts[:], rhs=ww_sb[l][:, 512:], start=True, stop=True)
                        hpA, hpB_s, hpB_d = None, hp2[:, 0:8], hp2[:, 8:16]
                    tt = wpool.tile([P, HC + 8], f32, tag="tt", name="tt")
                    if hpA is not None:
                        nc.vector.tensor_copy(tt[:], hpA)
                        sd = wpool.tile([P, 8], f32, tag="sd", name="sd")
                        nc.vector.tensor_copy(sd[:], hpB_s)
                    else:
                        nc.vector.tensor_copy(tt[:, :512], hp[:])
                        nc.vector.tensor_copy(tt[:, 512:520], hpB_s)
                        sd = wpool.tile([P, 8], f32, tag="sd", name="sd")
                        nc.vector.tensor_copy(sd[:], hpB_d)
                    nc.sync.dma_start(
                        out=tabA[l][i * P : i * P + cnt, : HC + 8], in_=tt[:cnt, :]
                    )
                    nc.sync.dma_start(out=tabS[i * P : i * P + cnt, :8], in_=sd[:cnt, :])

            # ================= edge phase =================
            def edge_phase(l, out_dram):
                cfg = LAYERS[l]
                C, R = cfg["C"], cfg["R"]
                HC = HEADS * C
                for t in range(NT):
                    cnt = min(P, NPC - t * P)
                    G = gpool.tile([P, NCH * R], f32, tag="G", name="G")
                    nc.gpsimd.dma_gather(
                        out_ap=G[:].rearrange("p (j r) -> p j r", r=R),
                        in_ap=tabA[l][:],
                        idxs_ap=isrc_sb[:, t * EPT // 16 : (t + 1) * EPT // 16],
                        num_idxs=EPT,
                        num_idxs_reg=ept_reg,
                        elem_size=R,
                        single_packet=False,
                    )
                    SD = gpool.tile([P, NCH * 64], f32, tag="SD", name="SD")
                    nc.gpsimd.dma_gather(
                        out_ap=SD[:].rearrange("p (j r) -> p j r", r=64),
                        in_ap=tabS[:],
                        idxs_ap=idst_sb[:, t * EPT // 16 : (t + 1) * EPT // 16],
                        num_idxs=EPT,
                        num_idxs_reg=ept_reg,
                        elem_size=64,
                        single_packet=False,
                    )
                    if HC + 8 <= 512:
                        psA = ppool.tile([P, HC + 8], f32, tag="pmm", name="psA", bufs=2)
                        psB = None
                    else:
                        psA = ppool.tile([P, 512], f32, tag="pmm", name="psA", bufs=2)
                        psB = ppool.tile([P, 8], f32, tag="pmm2", name="psB", bufs=2)
                    for j in range(NCH):
                        Gj = G[:, j * R : (j + 1) * R]
                        ebuf = wpool.tile([P, 8], f32, tag="ebuf", name="ebuf")
                        nc.vector.tensor_tensor(
                            ebuf[:], Gj[:, HC : HC + 8], SD[:, j * 64 : j * 64 + 8], OP.add
                        )
                        eb2 = wpool.tile([P, 8], f32, tag="eb2", name="eb2")
                        nc.vector.tensor_scalar(eb2[:], ebuf[:], NEG, None, OP.mult)
                        nc.vector.tensor_tensor(ebuf[:], ebuf[:], eb2[:], OP.max)
                        GEX = wpool.tile([P, HC + 8], f32, tag="GEX", name="GEX")
                        nc.scalar.activation(GEX[:, HC : HC + 8], ebuf[:], AF.Exp)
                        O = wpool.tile([P, P], f32, tag="O", name="O")
                        col = t * NCH + j
                        nc.vector.tensor_scalar(
                            O[:], iotaf_sb[:], dstloc_sb[:, col : col + 1], None, OP.is_equal
                        )
                        nc.vector.tensor_tensor(
                            GEX[:, :HC].rearrange("p (h c) -> p h c", h=HEADS),
                            Gj[:, :HC].rearrange("p (h c) -> p h c", h=HEADS),
                            GEX[:, HC : HC + 8].unsqueeze(2).to_broadcast((P, HEADS, C)),
                            OP.mult,
                        )
                        if psB is None:
                            nc.tensor.matmul(
                                out=psA[:], lhsT=O[:], rhs=GEX[:],
                                start=(j == 0), stop=(j == NCH - 1),
                            )
                        else:
                            nc.tensor.matmul(
                                out=psA[:], lhsT=O[:], rhs=GEX[:, :512],
                                start=(j == 0), stop=(j == NCH - 1),
                            )
                            nc.tensor.matmul(
                                out=psB[:], lhsT=O[:], rhs=GEX[:, 512:520],
                                start=(j == 0), stop=(j == NCH - 1),
                            )
                    den = psA[:, HC : HC + 8] if psB is None else psB[:]
                    rec = wpool.tile([P, 8], f32, tag="rec", name="rec")
                    nc.vector.tensor_scalar(rec[:], den, 1e-16, None, OP.add)
                    nc.vector.reciprocal(rec[:], rec[:])
                    res = wpool.tile([P, HC], f32, tag="res", name="res")
                    nc.vector.tensor_tensor(
                        res[:].rearrange("p (h c) -> p h c", h=HEADS),
                        psA[:, :HC].rearrange("p (h c) -> p h c", h=HEADS),
                        rec[:].unsqueeze(2).to_broadcast((P, HEADS, C)),
                        OP.mult,
                    )
                    if cfg["concat"]:
                        nc.vector.tensor_tensor(res[:], res[:], bb_sb[l][:], OP.add)
                        nc.sync.dma_start(
                            out=out_dram[t * P : t * P + cnt, :], in_=res[:cnt, :]
                        )
                    else:
                        red = wpool.tile([P, C], f32, tag="red", name="red")
                        nc.vector.tensor_reduce(
                            red[:],
                            res[:].rearrange("p (h c) -> p c h", h=HEADS),
                            AX,
                            OP.add,
                        )
                        nc.vector.tensor_scalar(red[:], red[:], 1.0 / HEADS, None, OP.mult)
                        nc.vector.tensor_tensor(red[:], red[:], bb_sb[l][:, :C], OP.add)
                        nc.sync.dma_start(
                            out=out_dram[t * P : t * P + cnt, :], in_=red[:cnt, :]
                        )

            # ================= full pipeline =================
            srcs = [xin, xg[0], xg[1], xg[2]]
            outs = [own[0], own[1], own[2], out_ext]
            def agather(l):
                if sim1:
                    # timeline-sim stand-in: keep the own->xg dependency edge
                    # without a collective (TimelineSim is single-core).
                    w = [128, 32, 128][l]
                    for i in range(NT):
                        cnt = min(P, NPC - i * P)
                        tb = wpool.tile([P, w], f32, tag="agsim", name="agsim")
                        nc.sync.dma_start(out=tb[:cnt, :], in_=own[l][i * P : i * P + cnt, :])
                        nc.sync.dma_start(out=xg[l][i * P : i * P + cnt, :], in_=tb[:cnt, :])
                    return
                nc.gpsimd.collective_compute(
                    "AllGather",
                    mybir.AluOpType.bypass,
                    replica_groups=[list(range(M))],
                    ins=[own[l].opt()],
                    outs=[xg[l].opt()],
                )
            if dbg == "tab0":
                node_phase(0, xin)
                hc8 = 136
                for i in range(NODE_TILES):
                    cnt = min(P, N - i * P)
                    tbuf = wpool.tile([P, 144], f32, tag="tbuf", name="tbuf")
                    nc.sync.dma_start(out=tbuf[:cnt, :hc8], in_=tabA[0][i*P:i*P+cnt, :hc8])
                    nc.sync.dma_start(out=tbuf[:cnt, hc8:144], in_=tabS[i*P:i*P+cnt, :8])
                    nc.sync.dma_start(out=out_ext[i*P:i*P+cnt, :], in_=tbuf[:cnt, :])
            elif dbg == "own0":
                node_phase(0, xin)
                edge_phase(0, out_ext)
            elif dbg == "xg0":
                node_phase(0, xin)
                edge_phase(0, own[0])
                agather(0)
                for i in range(NODE_TILES):
                    cnt = min(P, N - i * P)
                    tbuf = wpool.tile([P, 128], f32, tag="tbuf", name="tbuf")
                    nc.sync.dma_start(out=tbuf[:cnt, :], in_=xg[0][i*P:i*P+cnt, :])
                    nc.sync.dma_start(out=out_ext[i*P:i*P+cnt, :], in_=tbuf[:cnt, :])
            elif dbg == "tab1":
                node_phase(0, xin); edge_phase(0, own[0]); agather(0)
                node_phase(1, xg[0])
                hc8 = 264
                for i in range(NODE_TILES):
                    cnt = min(P, N - i * P)
                    tbuf = wpool.tile([P, 272], f32, tag="tbuf", name="tbuf")
                    nc.sync.dma_start(out=tbuf[:cnt, :hc8], in_=tabA[1][i*P:i*P+cnt, :hc8])
                    nc.sync.dma_start(out=tbuf[:cnt, hc8:272], in_=tabS[i*P:i*P+cnt, :8])
                    nc.sync.dma_start(out=out_ext[i*P:i*P+cnt, :], in_=tbuf[:cnt, :])
            elif dbg == "own1":
                node_phase(0, xin); edge_phase(0, own[0]); agather(0)
                node_phase(1, xg[0]); edge_phase(1, out_ext)
            else:
                for _rep in range(repeat_k):
                    if _rep > 0:
                        # Shared tensors allow a single writing instruction;
                        # later reps gather into their own buffers.
                        for l, w in [(0, 128), (1, 32), (2, 128)]:
                            xg[l] = dram.tile([N, w], f32, tag=f"xg{l}r{_rep}",
                                              name=f"xg{l}r{_rep}", addr_space="Shared")
                        srcs = [xin, xg[0], xg[1], xg[2]]
                    for l in range(4):
                        node_phase(l, srcs[l])
                        edge_phase(l, outs[l])
                        if l < 3:
                            agather(l)
    if not nc.is_finalized():
        nc.finalize()
    return nc


def _pjrt_exec(nc, in_maps, time_reps=0):
    """Mirror of bass2jax.run_bass_via_pjrt multi-core path, holding the jitted
    executable so repeated executions can be wall-timed (NTFF profiling is
    unavailable in this container)."""
    import time as _t
    import jax
    from jax.experimental.shard_map import shard_map
    from jax.sharding import Mesh, PartitionSpec
    from concourse import bass2jax as B, mybir as mb

    B.install_neuronx_cc_hook()
    n_cores = len(in_maps)
    partition_name = nc.partition_id_tensor.name if nc.partition_id_tensor else None
    in_names, out_names, out_avals, zero_outs = [], [], [], []
    for alloc in nc.m.functions[0].allocations:
        if not isinstance(alloc, mb.MemoryLocationSet):
            continue
        name = alloc.memorylocations[0].name
        if alloc.kind == "ExternalInput":
            if name != partition_name:
                in_names.append(name)
        elif alloc.kind == "ExternalOutput":
            out_names.append(name)
            shape = tuple(alloc.tensor_shape)
            dtype = mb.dt.np(alloc.dtype)
            out_avals.append(jax.core.ShapedArray(shape, dtype))
            zero_outs.append(np.zeros(shape, dtype))
    n_params = len(in_names)
    n_outs = len(out_avals)
    in_names.extend(out_names)
    if partition_name is not None:
        in_names.append(partition_name)
    donate = tuple(range(n_params, n_params + n_outs))

    def _body(*args):
        operands = list(args)
        if partition_name is not None:
            operands.append(B.partition_id_tensor())
        outs = B._bass_exec_p.bind(
            *operands,
            out_avals=tuple(out_avals),
            in_names=tuple(in_names),
            out_names=tuple(out_names),
            lowering_input_output_aliases=(),
            sim_require_finite=True,
            sim_require_nnan=True,
            nc=nc,
        )
        return tuple(outs)

    devices = jax.devices()[:n_cores]
    mesh = Mesh(np.asarray(devices), ("core",))
    in_specs = (PartitionSpec("core"),) * (n_params + n_outs)
    out_specs = (PartitionSpec("core"),) * len(out_names)
    sharded = jax.jit(
        shard_map(_body, mesh=mesh, in_specs=in_specs, out_specs=out_specs,
                  check_rep=False),
        donate_argnums=donate, keep_unused=True,
    )
    per_core = [[np.asarray(m_[nm]) for nm in in_names[:n_params]] for m_ in in_maps]
    concat_in = [
        np.concatenate([per_core[c][i] for c in range(n_cores)], axis=0)
        for i in range(n_params)
    ]
    from jax.sharding import NamedSharding
    shard = NamedSharding(mesh, PartitionSpec("core"))
    concat_in = [jax.device_put(a, shard) for a in concat_in]
    jax.block_until_ready(concat_in)

    def once():
        cz = [jax.device_put(np.zeros((n_cores * z.shape[0], *z.shape[1:]), z.dtype), shard)
              for z in zero_outs]
        jax.block_until_ready(cz)
        t0 = _t.perf_counter()
        out_arrs = sharded(*concat_in, *cz)
        jax.block_until_ready(out_arrs)
        return _t.perf_counter() - t0, out_arrs

    _, out_arrs = once()  # compile + first run
    times = []
    for _ in range(time_reps):
        dt, out_arrs = once()
        times.append(dt)
    res = [
        {nm: np.asarray(out_arrs[i]).reshape(n_cores, *out_avals[i].shape)[c]
         for i, nm in enumerate(out_names)}
        for c in range(n_cores)
    ]
    return res, (min(times) if times else None)


def _run(inputs, trace=False, time_reps=0, dbg=None, repeat_k=1):
    NCH, edata = _preprocess(np.asarray(inputs["edge_index"]))
    consts = _host_consts(inputs)
    nc = _build(NCH, dbg=dbg, repeat_k=repeat_k)

    in_maps = []
    for m in range(M):
        d = dict(consts)
        d.update(edata[m])
        in_maps.append(d)

    if time_reps > 0:
        results, best_s = _pjrt_exec(nc, in_maps, time_reps=time_reps)
    else:
        from concourse.bass_utils import run_bass_kernel_spmd

        res = run_bass_kernel_spmd(nc, in_maps, core_ids=list(range(M)))
        results, best_s = res.results, None
    outs = [np.asarray(results[m]["out"]) for m in range(M)]
    full = np.concatenate(outs, axis=0).astype(np.float32)
    return full, (None if best_s is None else int(best_s * 1e9))


def kernel(**inputs):
    out, _ = _run(inputs, trace=False)
    return out

